# revision 2
# baseline (speedup 1.0000x reference)
"""Trainium2 Bass kernel for nn_ConstraintLoss (segment_reduce).

Strategy
--------
The reference computes, over a sparse COO matrix A (16M nnz, sorted row ids):

    values = pred * (var_ub - var_lb) + var_lb          # (n_vars,)
    ax     = segment_sum(coeff * values[var_idx], constr_idx)
    viol   = per-sense relu/abs masking of (ax - rhs)
    out    = viol.mean()

Sharding / layout choice (host side, index plumbing only — no float math):
segments (constraint rows) are grouped into even-length classes (len padded
up to the class length L with zero slots), dealt round-robin to the 8 cores,
and each core's nnz are laid out into a dense [128, CW] slot grid where each
segment occupies L consecutive slots of one partition row.  The per-element
operands (coeff, pred[var_idx], lb[var_idx], ub[var_idx]) are placed into
that grid by pure index scatter/gather on host.  With this layout the whole
device computation is affine streaming:

    per [128, W] chunk:  w = ub - lb; t = pred*w; v = t + lb; prod = v*coeff
    per class range:     sums[:, cols] = reduce(prod viewed [128, K, L], X)
    finally:             diff = sums - rhs
                         viol = relu(diff)*a + relu(-diff)*b   (a, b from sense)
                         partial = sum(viol)   -> matmul with ones -> scalar

Each core owns a disjoint set of segments, so the cross-core reduction is
just the sum of 8 scalars (host unshard step), divided by n_constrs.
"""

import sys

if "/opt/trn_rl_repo" not in sys.path:
    sys.path.insert(0, "/opt/trn_rl_repo")

import numpy as np

N_CORES = 8
P = 128
WMAX = 2048  # max free-dim width (f32 elements) of a working chunk


# --------------------------------------------------------------------------
# Host-side layout planning (pure index math)
# --------------------------------------------------------------------------

def _plan(constr_idx, n_constrs):
    """Assign every segment to (core, partition, column) of an even-length
    class grid. Returns per-segment placement arrays + per-class metadata."""
    nnz = constr_idx.size
    ci = constr_idx.astype(np.int64)
    lens = np.bincount(ci, minlength=n_constrs).astype(np.int64)
    rp = np.zeros(n_constrs + 1, np.int64)
    np.cumsum(lens, out=rp[1:])

    # class length per segment: ceil to even, min 2; lengths in (32, 64] -> 64,
    # beyond that ceil to power of two (data-driven; unused classes don't exist)
    L_seg = np.maximum(2, ((lens + 1) // 2) * 2)
    big = L_seg > 32
    if np.any(big):
        bl = L_seg[big]
        pow2 = np.power(2, np.ceil(np.log2(bl)).astype(np.int64))
        L_seg[big] = np.maximum(64, pow2)
    assert L_seg.max() <= 1 << 14, f"segment too long: {lens.max()}"

    order = np.argsort(L_seg, kind="stable")
    L_sorted = L_seg[order]
    class_vals, class_starts, class_counts = np.unique(
        L_sorted, return_index=True, return_counts=True
    )

    seg_core = np.empty(n_constrs, np.int64)
    seg_slotbase = np.empty(n_constrs, np.int64)  # within-core flat [128*CW] addr
    seg_p = np.empty(n_constrs, np.int64)
    seg_col = np.empty(n_constrs, np.int64)  # column in sums [128, TOTCOL]

    meta = []  # (L, K, cw_off, col_off) per class
    cw_off = 0  # column offset in the slot grid (per partition row)
    col_off = 0  # column offset in the sums grid
    for Lc, st, cnt in zip(class_vals, class_starts, class_counts):
        Lc = int(Lc)
        segs = order[st : st + cnt]
        m = -(-int(cnt) // N_CORES)  # per-core segment count (ceil)
        K = -(-m // P)  # columns of this class
        j = np.arange(cnt, dtype=np.int64)
        core = j // m
        jj = j % m
        p = jj // K
        col = jj % K
        seg_core[segs] = core
        seg_p[segs] = p
        seg_col[segs] = col_off + col
        seg_slotbase[segs] = (cw_off + col * Lc) + p * 0  # placeholder, fixed below
        # within-core flat address of slot 0 of the segment: p*CW + cw_off + col*L
        # CW not known yet; store pieces and finish after the loop.
        seg_slotbase[segs] = col * Lc  # temp: column-local offset
        meta.append((Lc, K, cw_off, col_off))
        cw_off += K * Lc
        col_off += K
    CW = cw_off
    TOTCOL = col_off
    # finish slot base: p*CW + cw_off_class + col*L
    cls_cw_off = np.empty(n_constrs, np.int64)
    for (Lc, K, cwo, _), st, cnt in zip(meta, class_starts, class_counts):
        segs = order[st : st + cnt]
        cls_cw_off[segs] = cwo
    seg_slotbase = seg_p * CW + cls_cw_off + seg_slotbase

    return {
        "lens": lens,
        "rp": rp,
        "meta": meta,
        "CW": CW,
        "TOTCOL": TOTCOL,
        "seg_core": seg_core,
        "seg_slotbase": seg_slotbase,
        "seg_p": seg_p,
        "seg_col": seg_col,
    }


def _build_in_maps(plan, pred, coeff, constr_rhs, var_lb, var_ub, constr_idx,
                   var_idx, constr_sense, n_constrs, dt_np):
    """Scatter inputs into the per-core slot grids (index ops only)."""
    nnz = coeff.size
    CW, TOTCOL = plan["CW"], plan["TOTCOL"]
    ci = constr_idx.astype(np.int64)
    dst = (
        plan["seg_core"][ci] * (P * CW)
        + plan["seg_slotbase"][ci]
        + (np.arange(nnz, dtype=np.int64) - plan["rp"][ci])
    )
    vi = var_idx.astype(np.int64)

    def grid(src_vals):
        g = np.zeros(N_CORES * P * CW, dt_np)
        g[dst] = src_vals.astype(dt_np, copy=False)
        return g.reshape(N_CORES, P, CW)

    coeff_g = grid(coeff)
    pred_g = grid(pred[vi])
    lb_g = grid(var_lb[vi])
    ub_g = grid(var_ub[vi])

    saddr = (
        plan["seg_core"] * (P * TOTCOL)
        + plan["seg_p"] * TOTCOL
        + plan["seg_col"]
    )
    rhs_g = np.zeros(N_CORES * P * TOTCOL, np.float32)
    a_g = np.zeros(N_CORES * P * TOTCOL, np.float32)
    b_g = np.zeros(N_CORES * P * TOTCOL, np.float32)
    sense = constr_sense.astype(np.int64)
    rhs_g[saddr] = constr_rhs.astype(np.float32, copy=False)
    a_g[saddr] = ((sense == 1) | (sense == 3)).astype(np.float32)
    b_g[saddr] = -((sense == 2) | (sense == 3)).astype(np.float32)
    rhs_g = rhs_g.reshape(N_CORES, P, TOTCOL)
    a_g = a_g.reshape(N_CORES, P, TOTCOL)
    b_g = b_g.reshape(N_CORES, P, TOTCOL)

    in_maps = []
    for k in range(N_CORES):
        in_maps.append(
            {
                "coeff": coeff_g[k],
                "pred": pred_g[k],
                "lb": lb_g[k],
                "ub": ub_g[k],
                "rhs": rhs_g[k],
                "amask": a_g[k],
                "bneg": b_g[k],
            }
        )
    return in_maps


# --------------------------------------------------------------------------
# Device program
# --------------------------------------------------------------------------

def _build_nc(meta, CW, TOTCOL, dt):
    from concourse import bacc, bass, mybir, tile

    nc = bacc.Bacc("TRN2", target_bir_lowering=False)
    f32 = mybir.dt.float32

    coeff_d = nc.declare_dram_parameter("coeff", [P, CW], dt, isOutput=False)
    pred_d = nc.declare_dram_parameter("pred", [P, CW], dt, isOutput=False)
    lb_d = nc.declare_dram_parameter("lb", [P, CW], dt, isOutput=False)
    ub_d = nc.declare_dram_parameter("ub", [P, CW], dt, isOutput=False)
    rhs_d = nc.declare_dram_parameter("rhs", [P, TOTCOL], f32, isOutput=False)
    a_d = nc.declare_dram_parameter("amask", [P, TOTCOL], f32, isOutput=False)
    b_d = nc.declare_dram_parameter("bneg", [P, TOTCOL], f32, isOutput=False)
    out_d = nc.declare_dram_parameter("out", [1, 1], f32, isOutput=True)

    # chunk plan: walk classes, pack column ranges into <=WMAX-wide chunks.
    # each chunk: (cw0, W, [(L, Kpiece, rel_off_in_chunk, sums_col)])
    chunks = []
    cur = None  # [cw0, W, pieces]
    for (Lc, K, cwo, colo) in meta:
        kdone = 0
        while kdone < K:
            if cur is None:
                cur = [cwo + kdone * Lc, 0, []]
            room = (WMAX - cur[1]) // Lc
            if room == 0:
                chunks.append(cur)
                cur = [cwo + kdone * Lc, 0, []]
                room = WMAX // Lc
            take = min(K - kdone, room)
            cur[2].append((Lc, take, cur[1], colo + kdone))
            cur[1] += take * Lc
            kdone += take
    if cur is not None and cur[1] > 0:
        chunks.append(cur)

    ax_X = mybir.AxisListType.X
    op = mybir.AluOpType

    with tile.TileContext(nc) as tc:
        with (
            tc.tile_pool(name="persist", bufs=1) as pp,
            tc.tile_pool(name="work", bufs=3) as wp,
            tc.tile_pool(name="psum", bufs=1, space="PSUM") as psp,
        ):
            sums = pp.tile([P, TOTCOL], f32)

            for (cw0, W, pieces) in chunks:
                cf = wp.tile([P, W], dt)
                pr = wp.tile([P, W], dt)
                lbt = wp.tile([P, W], dt)
                ubt = wp.tile([P, W], dt)
                nc.sync.dma_start(out=cf[:], in_=coeff_d[:, cw0 : cw0 + W])
                nc.sync.dma_start(out=pr[:], in_=pred_d[:, cw0 : cw0 + W])
                nc.sync.dma_start(out=lbt[:], in_=lb_d[:, cw0 : cw0 + W])
                nc.sync.dma_start(out=ubt[:], in_=ub_d[:, cw0 : cw0 + W])
                nc.vector.tensor_sub(ubt[:], ubt[:], lbt[:])  # w = ub-lb
                nc.vector.tensor_mul(pr[:], pr[:], ubt[:])    # t = pred*w
                nc.vector.tensor_add(pr[:], pr[:], lbt[:])    # v = t+lb
                nc.vector.tensor_mul(cf[:], cf[:], pr[:])     # prod = v*coeff
                for (Lc, Kp, rel, scol) in pieces:
                    nc.vector.tensor_reduce(
                        out=sums[:, scol : scol + Kp],
                        in_=cf[:, rel : rel + Kp * Lc].rearrange(
                            "p (k l) -> p k l", l=Lc
                        ),
                        axis=ax_X,
                        op=op.add,
                    )

            rhs_t = pp.tile([P, TOTCOL], f32)
            a_t = pp.tile([P, TOTCOL], f32)
            b_t = pp.tile([P, TOTCOL], f32)
            nc.sync.dma_start(out=rhs_t[:], in_=rhs_d[:])
            nc.sync.dma_start(out=a_t[:], in_=a_d[:])
            nc.sync.dma_start(out=b_t[:], in_=b_d[:])

            acc = pp.tile([P, 1], f32)
            nc.vector.tensor_sub(sums[:], sums[:], rhs_t[:])  # diff
            # relu(diff)*a
            nc.vector.scalar_tensor_tensor(
                out=a_t[:], in0=sums[:], scalar=0.0, in1=a_t[:],
                op0=op.max, op1=op.mult,
            )
            # relu(-diff)*b  ==  min(diff,0) * (-b)
            nc.vector.scalar_tensor_tensor(
                out=b_t[:], in0=sums[:], scalar=0.0, in1=b_t[:],
                op0=op.min, op1=op.mult,
            )
            nc.vector.scalar_tensor_tensor(
                out=rhs_t[:], in0=a_t[:], scalar=0.0, in1=b_t[:],
                op0=op.add, op1=op.add, accum_out=acc[:],
            )

            ones = pp.tile([P, 1], f32)
            nc.vector.memset(ones[:], 1.0)
            ps = psp.tile([1, 1], f32)
            nc.tensor.matmul(out=ps[:], lhsT=ones[:], rhs=acc[:], start=True, stop=True)
            res = pp.tile([1, 1], f32)
            nc.vector.tensor_copy(out=res[:], in_=ps[:])
            nc.sync.dma_start(out=out_d[:], in_=res[:])

    nc.compile()
    return nc


# --------------------------------------------------------------------------
# PJRT runner (keeps the jitted executable for repeated timed runs)
# --------------------------------------------------------------------------

class _Runner:
    def __init__(self, nc):
        import jax
        import jax.numpy as jnp  # noqa: F401
        from jax.sharding import Mesh, PartitionSpec
        from jax.experimental.shard_map import shard_map
        from concourse import mybir
        from concourse.bass2jax import _bass_exec_p, install_neuronx_cc_hook

        install_neuronx_cc_hook()
        self.jax = jax

        partition_name = (
            nc.partition_id_tensor.name if nc.partition_id_tensor else None
        )
        in_names, out_names, out_avals, zero_outs = [], [], [], []
        for alloc in nc.m.functions[0].allocations:
            if not isinstance(alloc, mybir.MemoryLocationSet):
                continue
            name = alloc.memorylocations[0].name
            if alloc.kind == "ExternalInput":
                if name != partition_name:
                    in_names.append(name)
            elif alloc.kind == "ExternalOutput":
                out_names.append(name)
                shape = tuple(alloc.tensor_shape)
                dtype = mybir.dt.np(alloc.dtype)
                out_avals.append(jax.core.ShapedArray(shape, dtype))
                zero_outs.append(np.zeros(shape, dtype))
        n_params = len(in_names)
        n_outs = len(out_avals)
        all_in_names = list(in_names) + list(out_names)
        if partition_name is not None:
            all_in_names.append(partition_name)

        def _body(*args):
            operands = list(args)
            if partition_name is not None:
                from concourse.bass2jax import partition_id_tensor

                operands.append(partition_id_tensor())
            outs = _bass_exec_p.bind(
                *operands,
                out_avals=tuple(out_avals),
                in_names=tuple(all_in_names),
                out_names=tuple(out_names),
                lowering_input_output_aliases=(),
                sim_require_finite=True,
                sim_require_nnan=True,
                nc=nc,
            )
            return tuple(outs)

        devices = jax.devices()[:N_CORES]
        mesh = Mesh(np.asarray(devices), ("core",))
        in_specs = (PartitionSpec("core"),) * (n_params + n_outs)
        out_specs = (PartitionSpec("core"),) * n_outs
        donate = tuple(range(n_params, n_params + n_outs))
        self.sharded = jax.jit(
            shard_map(
                _body, mesh=mesh, in_specs=in_specs, out_specs=out_specs,
                check_rep=False,
            ),
            donate_argnums=donate,
            keep_unused=True,
        )
        self.in_names = in_names
        self.out_names = out_names
        self.zero_outs = zero_outs
        self.n_params = n_params

    def stage(self, in_maps):
        """Concatenate per-core inputs and push to device once."""
        concat = [
            np.ascontiguousarray(
                np.concatenate([m[name] for m in in_maps], axis=0)
            )
            for name in self.in_names
        ]
        self.staged = [self.jax.device_put(a) for a in concat]
        for a in self.staged:
            a.block_until_ready()

    def run(self):
        zeros = [
            np.zeros((N_CORES * z.shape[0], *z.shape[1:]), z.dtype)
            for z in self.zero_outs
        ]
        outs = self.sharded(*self.staged, *zeros)
        outs = [np.asarray(o) for o in outs]
        per_core = []
        for k in range(N_CORES):
            d = {}
            for i, name in enumerate(self.out_names):
                d[name] = outs[i].reshape(
                    N_CORES, *self.zero_outs[i].shape
                )[k]
            per_core.append(d)
        return per_core


_CACHE = {}


def _get_runner(meta_key, meta, CW, TOTCOL, dt_name):
    if meta_key in _CACHE:
        return _CACHE[meta_key]
    from concourse import mybir

    dt = getattr(mybir.dt, dt_name)
    nc = _build_nc(meta, CW, TOTCOL, dt)
    runner = _Runner(nc)
    _CACHE[meta_key] = runner
    return runner


# --------------------------------------------------------------------------
# Entry point
# --------------------------------------------------------------------------

def kernel(pred, coeff, constr_rhs, var_lb, var_ub, constr_idx, var_idx,
           constr_sense, n_vars, n_constrs, **_unused):
    n_constrs = int(n_constrs)
    n_vars = int(n_vars)
    pred = np.asarray(pred)
    coeff = np.asarray(coeff)
    constr_rhs = np.asarray(constr_rhs)
    var_lb = np.asarray(var_lb)
    var_ub = np.asarray(var_ub)
    constr_idx = np.asarray(constr_idx)
    var_idx = np.asarray(var_idx)
    constr_sense = np.asarray(constr_sense)

    plan = _plan(constr_idx, n_constrs)
    in_maps = _build_in_maps(
        plan, pred, coeff, constr_rhs, var_lb, var_ub, constr_idx, var_idx,
        constr_sense, n_constrs, np.float32,
    )
    meta_key = (tuple(plan["meta"]), plan["CW"], plan["TOTCOL"], "float32")
    runner = _get_runner(meta_key, plan["meta"], plan["CW"], plan["TOTCOL"],
                         "float32")
    runner.stage(in_maps)
    results = runner.run()
    total = sum(float(r["out"][0, 0]) for r in results)
    return np.float32(total / n_constrs)


# revision 9
# speedup vs baseline: 294.7736x; 294.7736x over previous
"""Trainium2 Bass kernel for nn_ConstraintLoss (segment_reduce).

Strategy
--------
The reference computes, over a sparse COO matrix A (16M nnz, sorted row ids):

    values = pred * (var_ub - var_lb) + var_lb          # (n_vars,)
    ax     = segment_sum(coeff * values[var_idx], constr_idx)
    viol   = per-sense relu/abs masking of (ax - rhs)
    out    = viol.mean()

Sharding / layout choice (host side, index plumbing only — no float math):
segments (constraint rows) are grouped into even-length classes (len padded
up to the class length L with zero slots), dealt round-robin to the 8 cores,
and each core's nnz are laid out into a dense [128, CW] slot grid where each
segment occupies L consecutive slots of one partition row.  The per-element
operands (coeff, pred[var_idx], lb[var_idx], ub[var_idx]) are placed into
that grid by pure index scatter/gather on host.  With this layout the whole
device computation is affine streaming:

    per [128, W] chunk:  w = ub - lb; t = pred*w; v = t + lb; prod = v*coeff
    per class range:     sums[:, cols] = reduce(prod viewed [128, K, L], X)
    finally:             diff = sums - rhs
                         viol = relu(diff)*a + relu(-diff)*b   (a, b from sense)
                         partial = sum(viol)   -> matmul with ones -> scalar

Each core owns a disjoint set of segments, so the cross-core reduction is
just the sum of 8 scalars (host unshard step), divided by n_constrs.
"""

import sys

if "/opt/trn_rl_repo" not in sys.path:
    sys.path.insert(0, "/opt/trn_rl_repo")

import numpy as np

N_CORES = 8
P = 128
WMAX = 2048  # max free-dim width (f32 elements) of a working chunk
DTYPE = "float32"  # "float32" | "bfloat16" for the streamed slot arrays


# --------------------------------------------------------------------------
# Host-side layout planning (pure index math)
# --------------------------------------------------------------------------

def _plan(constr_idx, n_constrs):
    """Assign every segment to (core, partition, column) of an even-length
    class grid. Returns per-segment placement arrays + per-class metadata."""
    nnz = constr_idx.size
    ci = constr_idx.astype(np.int64)
    lens = np.bincount(ci, minlength=n_constrs).astype(np.int64)
    rp = np.zeros(n_constrs + 1, np.int64)
    np.cumsum(lens, out=rp[1:])

    # class length per segment: ceil to even, min 2; lengths in (32, 64] -> 64,
    # beyond that ceil to power of two (data-driven; unused classes don't exist)
    L_seg = np.maximum(2, ((lens + 1) // 2) * 2)
    big = L_seg > 32
    if np.any(big):
        bl = L_seg[big]
        pow2 = np.power(2, np.ceil(np.log2(bl)).astype(np.int64))
        L_seg[big] = np.maximum(64, pow2)
    assert L_seg.max() <= 1 << 14, f"segment too long: {lens.max()}"

    order = np.argsort(L_seg, kind="stable")
    L_sorted = L_seg[order]
    class_vals, class_starts, class_counts = np.unique(
        L_sorted, return_index=True, return_counts=True
    )

    seg_core = np.empty(n_constrs, np.int64)
    seg_slotbase = np.empty(n_constrs, np.int64)  # within-core flat [128*CW] addr
    seg_p = np.empty(n_constrs, np.int64)
    seg_col = np.empty(n_constrs, np.int64)  # column in sums [128, TOTCOL]

    meta = []  # (L, K, cw_off, col_off) per class
    cw_off = 0  # column offset in the slot grid (per partition row)
    col_off = 0  # column offset in the sums grid
    for Lc, st, cnt in zip(class_vals, class_starts, class_counts):
        Lc = int(Lc)
        segs = order[st : st + cnt]
        m = -(-int(cnt) // N_CORES)  # per-core segment count (ceil)
        K = -(-m // P)  # columns of this class
        j = np.arange(cnt, dtype=np.int64)
        core = j // m
        jj = j % m
        p = jj // K
        col = jj % K
        seg_core[segs] = core
        seg_p[segs] = p
        seg_col[segs] = col_off + col
        seg_slotbase[segs] = (cw_off + col * Lc) + p * 0  # placeholder, fixed below
        # within-core flat address of slot 0 of the segment: p*CW + cw_off + col*L
        # CW not known yet; store pieces and finish after the loop.
        seg_slotbase[segs] = col * Lc  # temp: column-local offset
        meta.append((Lc, K, cw_off, col_off))
        cw_off += K * Lc
        col_off += K
    CW = cw_off
    TOTCOL = col_off
    # finish slot base: p*CW + cw_off_class + col*L
    cls_cw_off = np.empty(n_constrs, np.int64)
    for (Lc, K, cwo, _), st, cnt in zip(meta, class_starts, class_counts):
        segs = order[st : st + cnt]
        cls_cw_off[segs] = cwo
    seg_slotbase = seg_p * CW + cls_cw_off + seg_slotbase

    return {
        "lens": lens,
        "rp": rp,
        "meta": meta,
        "CW": CW,
        "TOTCOL": TOTCOL,
        "seg_core": seg_core,
        "seg_slotbase": seg_slotbase,
        "seg_p": seg_p,
        "seg_col": seg_col,
    }


def _build_in_maps(plan, pred, coeff, constr_rhs, var_lb, var_ub, constr_idx,
                   var_idx, constr_sense, n_constrs, dt_np):
    """Scatter inputs into the per-core slot grids (index ops only)."""
    nnz = coeff.size
    CW, TOTCOL = plan["CW"], plan["TOTCOL"]
    ci = constr_idx.astype(np.int64)
    dst = (
        plan["seg_core"][ci] * (P * CW)
        + plan["seg_slotbase"][ci]
        + (np.arange(nnz, dtype=np.int64) - plan["rp"][ci])
    )
    vi = var_idx.astype(np.int64)

    def grid(src_vals):
        g = np.zeros(N_CORES * P * CW, dt_np)
        g[dst] = src_vals.astype(dt_np, copy=False)
        return g.reshape(N_CORES, P, CW)

    coeff_g = grid(coeff)
    pred_g = grid(pred[vi])
    lb_g = grid(var_lb[vi])
    ub_g = grid(var_ub[vi])

    saddr = (
        plan["seg_core"] * (P * TOTCOL)
        + plan["seg_p"] * TOTCOL
        + plan["seg_col"]
    )
    rhs_g = np.zeros(N_CORES * P * TOTCOL, np.float32)
    a_g = np.zeros(N_CORES * P * TOTCOL, np.float32)
    b_g = np.zeros(N_CORES * P * TOTCOL, np.float32)
    sense = constr_sense.astype(np.int64)
    rhs_g[saddr] = constr_rhs.astype(np.float32, copy=False)
    a_g[saddr] = ((sense == 1) | (sense == 3)).astype(np.float32)
    b_g[saddr] = -((sense == 2) | (sense == 3)).astype(np.float32)
    rhs_g = rhs_g.reshape(N_CORES, P, TOTCOL)
    a_g = a_g.reshape(N_CORES, P, TOTCOL)
    b_g = b_g.reshape(N_CORES, P, TOTCOL)

    in_maps = []
    for k in range(N_CORES):
        in_maps.append(
            {
                "coeff": coeff_g[k],
                "pred": pred_g[k],
                "lb": lb_g[k],
                "ub": ub_g[k],
                "rhs": rhs_g[k],
                "amask": a_g[k],
                "bneg": b_g[k],
            }
        )
    return in_maps


# --------------------------------------------------------------------------
# Device program
# --------------------------------------------------------------------------

def _build_nc(meta, CW, TOTCOL, dt, repeat=1):
    from concourse import bacc, bass, mybir, tile
    from contextlib import ExitStack

    nc = bacc.Bacc("TRN2", target_bir_lowering=False)
    f32 = mybir.dt.float32

    coeff_d = nc.declare_dram_parameter("coeff", [P, CW], dt, isOutput=False)
    pred_d = nc.declare_dram_parameter("pred", [P, CW], dt, isOutput=False)
    lb_d = nc.declare_dram_parameter("lb", [P, CW], dt, isOutput=False)
    ub_d = nc.declare_dram_parameter("ub", [P, CW], dt, isOutput=False)
    rhs_d = nc.declare_dram_parameter("rhs", [P, TOTCOL], f32, isOutput=False)
    a_d = nc.declare_dram_parameter("amask", [P, TOTCOL], f32, isOutput=False)
    b_d = nc.declare_dram_parameter("bneg", [P, TOTCOL], f32, isOutput=False)
    out_d = nc.declare_dram_parameter("out", [1, 1], f32, isOutput=True)

    # chunk plan: walk classes, pack column ranges into <=WMAX-wide chunks.
    # each chunk: (cw0, W, [(L, Kpiece, rel_off_in_chunk, sums_col)])
    chunks = []
    cur = None  # [cw0, W, pieces]
    for (Lc, K, cwo, colo) in meta:
        kdone = 0
        while kdone < K:
            if cur is None:
                cur = [cwo + kdone * Lc, 0, []]
            room = (WMAX - cur[1]) // Lc
            if room == 0:
                chunks.append(cur)
                cur = [cwo + kdone * Lc, 0, []]
                room = WMAX // Lc
            take = min(K - kdone, room)
            cur[2].append((Lc, take, cur[1], colo + kdone))
            cur[1] += take * Lc
            kdone += take
    if cur is not None and cur[1] > 0:
        chunks.append(cur)

    ax_X = mybir.AxisListType.X
    op = mybir.AluOpType

    with tile.TileContext(nc) as tc:
        with (
            tc.tile_pool(name="persist", bufs=1) as pp,
            tc.tile_pool(name="work", bufs=3) as wp,
            tc.tile_pool(name="psum", bufs=1, space="PSUM") as psp,
            ExitStack() as stk,
        ):
            sums = pp.tile([P, TOTCOL], f32)
            if repeat > 1:
                stk.enter_context(tc.For_i(0, repeat))

            for (cw0, W, pieces) in chunks:
                cf = wp.tile([P, W], dt)
                pr = wp.tile([P, W], dt)
                lbt = wp.tile([P, W], dt)
                ubt = wp.tile([P, W], dt)
                nc.sync.dma_start(out=cf[:], in_=coeff_d[:, cw0 : cw0 + W])
                nc.sync.dma_start(out=pr[:], in_=pred_d[:, cw0 : cw0 + W])
                nc.sync.dma_start(out=lbt[:], in_=lb_d[:, cw0 : cw0 + W])
                nc.sync.dma_start(out=ubt[:], in_=ub_d[:, cw0 : cw0 + W])
                nc.vector.tensor_sub(ubt[:], ubt[:], lbt[:])  # w = ub-lb
                nc.vector.tensor_mul(pr[:], pr[:], ubt[:])    # t = pred*w
                nc.vector.tensor_add(pr[:], pr[:], lbt[:])    # v = t+lb
                nc.vector.tensor_mul(cf[:], cf[:], pr[:])     # prod = v*coeff
                for (Lc, Kp, rel, scol) in pieces:
                    nc.vector.tensor_reduce(
                        out=sums[:, scol : scol + Kp],
                        in_=cf[:, rel : rel + Kp * Lc].rearrange(
                            "p (k l) -> p k l", l=Lc
                        ),
                        axis=ax_X,
                        op=op.add,
                    )

            rhs_t = pp.tile([P, TOTCOL], f32)
            a_t = pp.tile([P, TOTCOL], f32)
            b_t = pp.tile([P, TOTCOL], f32)
            nc.sync.dma_start(out=rhs_t[:], in_=rhs_d[:])
            nc.sync.dma_start(out=a_t[:], in_=a_d[:])
            nc.sync.dma_start(out=b_t[:], in_=b_d[:])

            acc = pp.tile([P, 1], f32)
            nc.vector.tensor_sub(sums[:], sums[:], rhs_t[:])  # diff
            # relu(diff)*a
            nc.vector.scalar_tensor_tensor(
                out=a_t[:], in0=sums[:], scalar=0.0, in1=a_t[:],
                op0=op.max, op1=op.mult,
            )
            # relu(-diff)*b  ==  min(diff,0) * (-b)
            nc.vector.scalar_tensor_tensor(
                out=b_t[:], in0=sums[:], scalar=0.0, in1=b_t[:],
                op0=op.min, op1=op.mult,
            )
            nc.vector.scalar_tensor_tensor(
                out=rhs_t[:], in0=a_t[:], scalar=0.0, in1=b_t[:],
                op0=op.add, op1=op.add, accum_out=acc[:],
            )

            ones = pp.tile([P, 1], f32)
            nc.vector.memset(ones[:], 1.0)
            ps = psp.tile([1, 1], f32)
            nc.tensor.matmul(out=ps[:], lhsT=ones[:], rhs=acc[:], start=True, stop=True)
            res = pp.tile([1, 1], f32)
            nc.vector.tensor_copy(out=res[:], in_=ps[:])
            nc.sync.dma_start(out=out_d[:], in_=res[:])

    nc.compile()
    return nc


# --------------------------------------------------------------------------
# PJRT runner (keeps the jitted executable for repeated timed runs)
# --------------------------------------------------------------------------

class _Runner:
    def __init__(self, nc):
        import jax
        import jax.numpy as jnp  # noqa: F401
        from jax.sharding import Mesh, PartitionSpec
        from jax.experimental.shard_map import shard_map
        from concourse import mybir
        from concourse.bass2jax import _bass_exec_p, install_neuronx_cc_hook

        install_neuronx_cc_hook()
        self.jax = jax

        partition_name = (
            nc.partition_id_tensor.name if nc.partition_id_tensor else None
        )
        in_names, out_names, out_avals, zero_outs = [], [], [], []
        for alloc in nc.m.functions[0].allocations:
            if not isinstance(alloc, mybir.MemoryLocationSet):
                continue
            name = alloc.memorylocations[0].name
            if alloc.kind == "ExternalInput":
                if name != partition_name:
                    in_names.append(name)
            elif alloc.kind == "ExternalOutput":
                out_names.append(name)
                shape = tuple(alloc.tensor_shape)
                dtype = mybir.dt.np(alloc.dtype)
                out_avals.append(jax.core.ShapedArray(shape, dtype))
                zero_outs.append(np.zeros(shape, dtype))
        n_params = len(in_names)
        n_outs = len(out_avals)
        all_in_names = list(in_names) + list(out_names)
        if partition_name is not None:
            all_in_names.append(partition_name)

        def _body(*args):
            operands = list(args)
            if partition_name is not None:
                from concourse.bass2jax import partition_id_tensor

                operands.append(partition_id_tensor())
            outs = _bass_exec_p.bind(
                *operands,
                out_avals=tuple(out_avals),
                in_names=tuple(all_in_names),
                out_names=tuple(out_names),
                lowering_input_output_aliases=(),
                sim_require_finite=True,
                sim_require_nnan=True,
                nc=nc,
            )
            return tuple(outs)

        devices = jax.devices()[:N_CORES]
        mesh = Mesh(np.asarray(devices), ("core",))
        self.mesh = mesh
        in_specs = (PartitionSpec("core"),) * (n_params + n_outs)
        out_specs = (PartitionSpec("core"),) * n_outs
        donate = tuple(range(n_params, n_params + n_outs))
        self.sharded = jax.jit(
            shard_map(
                _body, mesh=mesh, in_specs=in_specs, out_specs=out_specs,
                check_rep=False,
            ),
            donate_argnums=donate,
            keep_unused=True,
        )
        self.in_names = in_names
        self.out_names = out_names
        self.zero_outs = zero_outs
        self.n_params = n_params

    def stage(self, in_maps):
        """Concatenate per-core inputs and push to device, sharded by core."""
        from jax.sharding import NamedSharding, PartitionSpec

        sh = NamedSharding(self.mesh, PartitionSpec("core"))
        concat = [
            np.ascontiguousarray(
                np.concatenate([m[name] for m in in_maps], axis=0)
            )
            for name in self.in_names
        ]
        self.staged = [self.jax.device_put(a, sh) for a in concat]
        for a in self.staged:
            a.block_until_ready()

    def run(self):
        zeros = [
            np.zeros((N_CORES * z.shape[0], *z.shape[1:]), z.dtype)
            for z in self.zero_outs
        ]
        outs = self.sharded(*self.staged, *zeros)
        outs = [np.asarray(o) for o in outs]
        per_core = []
        for k in range(N_CORES):
            d = {}
            for i, name in enumerate(self.out_names):
                d[name] = outs[i].reshape(
                    N_CORES, *self.zero_outs[i].shape
                )[k]
            per_core.append(d)
        return per_core


_CACHE = {}


def _get_runner(meta_key, meta, CW, TOTCOL, dt_name, repeat=1):
    key = (meta_key, repeat)
    if key in _CACHE:
        return _CACHE[key]
    from concourse import mybir

    dt = getattr(mybir.dt, dt_name)
    nc = _build_nc(meta, CW, TOTCOL, dt, repeat=repeat)
    runner = _Runner(nc)
    _CACHE[key] = runner
    return runner


# --------------------------------------------------------------------------
# Entry point
# --------------------------------------------------------------------------

def kernel(pred, coeff, constr_rhs, var_lb, var_ub, constr_idx, var_idx,
           constr_sense, n_vars, n_constrs, **_unused):
    n_constrs = int(n_constrs)
    n_vars = int(n_vars)
    pred = np.asarray(pred)
    coeff = np.asarray(coeff)
    constr_rhs = np.asarray(constr_rhs)
    var_lb = np.asarray(var_lb)
    var_ub = np.asarray(var_ub)
    constr_idx = np.asarray(constr_idx)
    var_idx = np.asarray(var_idx)
    constr_sense = np.asarray(constr_sense)

    if DTYPE == "float32":
        dt_np = np.float32
    else:
        import ml_dtypes

        dt_np = ml_dtypes.bfloat16

    plan = _plan(constr_idx, n_constrs)
    in_maps = _build_in_maps(
        plan, pred, coeff, constr_rhs, var_lb, var_ub, constr_idx, var_idx,
        constr_sense, n_constrs, dt_np,
    )
    meta_key = (tuple(plan["meta"]), plan["CW"], plan["TOTCOL"], DTYPE)
    runner = _get_runner(meta_key, plan["meta"], plan["CW"], plan["TOTCOL"],
                         DTYPE)
    runner.stage(in_maps)
    results = runner.run()
    total = sum(float(r["out"][0, 0]) for r in results)
    return np.float32(total / n_constrs)


# revision 11
# speedup vs baseline: 1763.1272x; 5.9813x over previous
"""Trainium2 Bass kernel for nn_ConstraintLoss (segment_reduce).

Strategy
--------
The reference computes, over a sparse COO matrix A (16M nnz, sorted row ids):

    values = pred * (var_ub - var_lb) + var_lb          # (n_vars,)
    ax     = segment_sum(coeff * values[var_idx], constr_idx)
    viol   = per-sense relu/abs masking of (ax - rhs)
    out    = viol.mean()

Sharding / layout choice (host side, index plumbing only — no float math):
segments (constraint rows) are grouped into even-length classes (len padded
up to the class length L with zero slots), dealt round-robin to the 8 cores,
and each core's nnz are laid out into a dense [128, CW] slot grid where each
segment occupies L consecutive slots of one partition row.  The per-element
operands (coeff, pred[var_idx], lb[var_idx], ub[var_idx]) are placed into
that grid by pure index scatter/gather on host.  With this layout the whole
device computation is affine streaming:

    per [128, W] chunk:  w = ub - lb; t = pred*w; v = t + lb; prod = v*coeff
    per class range:     sums[:, cols] = reduce(prod viewed [128, K, L], X)
    finally:             diff = sums - rhs
                         viol = relu(diff)*a + relu(-diff)*b   (a, b from sense)
                         partial = sum(viol)   -> matmul with ones -> scalar

Each core owns a disjoint set of segments, so the cross-core reduction is
just the sum of 8 scalars (host unshard step), divided by n_constrs.
"""

import sys

if "/opt/trn_rl_repo" not in sys.path:
    sys.path.insert(0, "/opt/trn_rl_repo")

import numpy as np

N_CORES = 8
P = 128
WMAX = 2048  # max free-dim width (f32 elements) of a working chunk
DTYPE = "float32"  # "float32" | "bfloat16" for the streamed slot arrays


# --------------------------------------------------------------------------
# Host-side layout planning (pure index math)
# --------------------------------------------------------------------------

def _plan(constr_idx, n_constrs):
    """Assign every segment to (core, partition, column) of an even-length
    class grid. Returns per-segment placement arrays + per-class metadata."""
    nnz = constr_idx.size
    ci = constr_idx.astype(np.int64)
    lens = np.bincount(ci, minlength=n_constrs).astype(np.int64)
    rp = np.zeros(n_constrs + 1, np.int64)
    np.cumsum(lens, out=rp[1:])

    # class length per segment: ceil to even, min 2; lengths in (32, 64] -> 64,
    # beyond that ceil to power of two (data-driven; unused classes don't exist)
    L_seg = np.maximum(2, ((lens + 1) // 2) * 2)
    big = L_seg > 32
    if np.any(big):
        bl = L_seg[big]
        pow2 = np.power(2, np.ceil(np.log2(bl)).astype(np.int64))
        L_seg[big] = np.maximum(64, pow2)
    assert L_seg.max() <= 1 << 14, f"segment too long: {lens.max()}"

    order = np.argsort(L_seg, kind="stable")
    L_sorted = L_seg[order]
    class_vals, class_starts, class_counts = np.unique(
        L_sorted, return_index=True, return_counts=True
    )

    seg_core = np.empty(n_constrs, np.int64)
    seg_slotbase = np.empty(n_constrs, np.int64)  # within-core flat [128*CW] addr
    seg_p = np.empty(n_constrs, np.int64)
    seg_col = np.empty(n_constrs, np.int64)  # column in sums [128, TOTCOL]

    meta = []  # (L, K, cw_off, col_off) per class
    cw_off = 0  # column offset in the slot grid (per partition row)
    col_off = 0  # column offset in the sums grid
    for Lc, st, cnt in zip(class_vals, class_starts, class_counts):
        Lc = int(Lc)
        segs = order[st : st + cnt]
        m = -(-int(cnt) // N_CORES)  # per-core segment count (ceil)
        K = -(-m // P)  # columns of this class
        j = np.arange(cnt, dtype=np.int64)
        core = j // m
        jj = j % m
        p = jj // K
        col = jj % K
        seg_core[segs] = core
        seg_p[segs] = p
        seg_col[segs] = col_off + col
        seg_slotbase[segs] = (cw_off + col * Lc) + p * 0  # placeholder, fixed below
        # within-core flat address of slot 0 of the segment: p*CW + cw_off + col*L
        # CW not known yet; store pieces and finish after the loop.
        seg_slotbase[segs] = col * Lc  # temp: column-local offset
        meta.append((Lc, K, cw_off, col_off))
        cw_off += K * Lc
        col_off += K
    CW = cw_off
    TOTCOL = col_off
    # finish slot base: p*CW + cw_off_class + col*L
    cls_cw_off = np.empty(n_constrs, np.int64)
    for (Lc, K, cwo, _), st, cnt in zip(meta, class_starts, class_counts):
        segs = order[st : st + cnt]
        cls_cw_off[segs] = cwo
    seg_slotbase = seg_p * CW + cls_cw_off + seg_slotbase

    return {
        "lens": lens,
        "rp": rp,
        "meta": meta,
        "CW": CW,
        "TOTCOL": TOTCOL,
        "seg_core": seg_core,
        "seg_slotbase": seg_slotbase,
        "seg_p": seg_p,
        "seg_col": seg_col,
    }


def _build_in_maps(plan, pred, coeff, constr_rhs, var_lb, var_ub, constr_idx,
                   var_idx, constr_sense, n_constrs, dt_np):
    """Scatter inputs into the per-core slot grids (index ops only)."""
    nnz = coeff.size
    CW, TOTCOL = plan["CW"], plan["TOTCOL"]
    ci = constr_idx.astype(np.int64)
    dst = (
        plan["seg_core"][ci] * (P * CW)
        + plan["seg_slotbase"][ci]
        + (np.arange(nnz, dtype=np.int64) - plan["rp"][ci])
    )
    vi = var_idx.astype(np.int64)

    def grid(src_vals):
        g = np.zeros(N_CORES * P * CW, dt_np)
        g[dst] = src_vals.astype(dt_np, copy=False)
        return g.reshape(N_CORES, P, CW)

    coeff_g = grid(coeff)
    pred_g = grid(pred[vi])
    lb_g = grid(var_lb[vi])
    ub_g = grid(var_ub[vi])

    saddr = (
        plan["seg_core"] * (P * TOTCOL)
        + plan["seg_p"] * TOTCOL
        + plan["seg_col"]
    )
    rhs_g = np.zeros(N_CORES * P * TOTCOL, np.float32)
    a_g = np.zeros(N_CORES * P * TOTCOL, np.float32)
    b_g = np.zeros(N_CORES * P * TOTCOL, np.float32)
    sense = constr_sense.astype(np.int64)
    rhs_g[saddr] = constr_rhs.astype(np.float32, copy=False)
    a_g[saddr] = ((sense == 1) | (sense == 3)).astype(np.float32)
    b_g[saddr] = -((sense == 2) | (sense == 3)).astype(np.float32)
    rhs_g = rhs_g.reshape(N_CORES, P, TOTCOL)
    a_g = a_g.reshape(N_CORES, P, TOTCOL)
    b_g = b_g.reshape(N_CORES, P, TOTCOL)

    in_maps = []
    for k in range(N_CORES):
        in_maps.append(
            {
                "coeff": coeff_g[k],
                "pred": pred_g[k],
                "lb": lb_g[k],
                "ub": ub_g[k],
                "rhs": rhs_g[k],
                "amask": a_g[k],
                "bneg": b_g[k],
            }
        )
    return in_maps


# --------------------------------------------------------------------------
# Device program
# --------------------------------------------------------------------------

def _build_nc(meta, CW, TOTCOL, dt, repeat=1, mode="full"):
    """mode: 'full' | 'dma' (loads only) | 'tt' (no reduce) | 'reduce'
    (coeff DMA + reduces only) — for bottleneck isolation."""
    from concourse import bacc, bass, mybir, tile
    from contextlib import ExitStack

    nc = bacc.Bacc("TRN2", target_bir_lowering=False)
    f32 = mybir.dt.float32

    coeff_d = nc.declare_dram_parameter("coeff", [P, CW], dt, isOutput=False)
    pred_d = nc.declare_dram_parameter("pred", [P, CW], dt, isOutput=False)
    lb_d = nc.declare_dram_parameter("lb", [P, CW], dt, isOutput=False)
    ub_d = nc.declare_dram_parameter("ub", [P, CW], dt, isOutput=False)
    rhs_d = nc.declare_dram_parameter("rhs", [P, TOTCOL], f32, isOutput=False)
    a_d = nc.declare_dram_parameter("amask", [P, TOTCOL], f32, isOutput=False)
    b_d = nc.declare_dram_parameter("bneg", [P, TOTCOL], f32, isOutput=False)
    out_d = nc.declare_dram_parameter("out", [1, 1], f32, isOutput=True)

    # chunk plan: walk classes, pack column ranges into <=WMAX-wide chunks.
    # each chunk: (cw0, W, [(L, Kpiece, rel_off_in_chunk, sums_col)])
    chunks = []
    cur = None  # [cw0, W, pieces]
    for (Lc, K, cwo, colo) in meta:
        kdone = 0
        while kdone < K:
            if cur is None:
                cur = [cwo + kdone * Lc, 0, []]
            room = (WMAX - cur[1]) // Lc
            if room == 0:
                chunks.append(cur)
                cur = [cwo + kdone * Lc, 0, []]
                room = WMAX // Lc
            take = min(K - kdone, room)
            cur[2].append((Lc, take, cur[1], colo + kdone))
            cur[1] += take * Lc
            kdone += take
    if cur is not None and cur[1] > 0:
        chunks.append(cur)

    ax_X = mybir.AxisListType.X
    op = mybir.AluOpType

    with tile.TileContext(nc) as tc:
        with (
            tc.tile_pool(name="persist", bufs=1) as pp,
            tc.tile_pool(name="work", bufs=3) as wp,
            tc.tile_pool(name="psum", bufs=1, space="PSUM") as psp,
            ExitStack() as stk,
        ):
            sums = pp.tile([P, TOTCOL], f32)
            if repeat > 1:
                stk.enter_context(tc.For_i(0, repeat))

            for (cw0, W, pieces) in chunks:
                cf = wp.tile([P, W], dt)
                nc.sync.dma_start(out=cf[:], in_=coeff_d[:, cw0 : cw0 + W])
                if mode in ("full", "dma", "tt"):
                    pr = wp.tile([P, W], dt)
                    lbt = wp.tile([P, W], dt)
                    ubt = wp.tile([P, W], dt)
                    nc.sync.dma_start(out=pr[:], in_=pred_d[:, cw0 : cw0 + W])
                    nc.sync.dma_start(out=lbt[:], in_=lb_d[:, cw0 : cw0 + W])
                    nc.sync.dma_start(out=ubt[:], in_=ub_d[:, cw0 : cw0 + W])
                if mode in ("full", "tt"):
                    nc.vector.tensor_sub(ubt[:], ubt[:], lbt[:])  # w = ub-lb
                    nc.vector.tensor_mul(pr[:], pr[:], ubt[:])    # t = pred*w
                    nc.vector.tensor_add(pr[:], pr[:], lbt[:])    # v = t+lb
                    nc.vector.tensor_mul(cf[:], cf[:], pr[:])     # prod = v*coeff
                if mode in ("full", "reduce"):
                    for (Lc, Kp, rel, scol) in pieces:
                        nc.vector.tensor_reduce(
                            out=sums[:, scol : scol + Kp],
                            in_=cf[:, rel : rel + Kp * Lc].rearrange(
                                "p (k l) -> p k l", l=Lc
                            ),
                            axis=ax_X,
                            op=op.add,
                        )

            rhs_t = pp.tile([P, TOTCOL], f32)
            a_t = pp.tile([P, TOTCOL], f32)
            b_t = pp.tile([P, TOTCOL], f32)
            nc.sync.dma_start(out=rhs_t[:], in_=rhs_d[:])
            nc.sync.dma_start(out=a_t[:], in_=a_d[:])
            nc.sync.dma_start(out=b_t[:], in_=b_d[:])

            acc = pp.tile([P, 1], f32)
            nc.vector.tensor_sub(sums[:], sums[:], rhs_t[:])  # diff
            # relu(diff)*a
            nc.vector.scalar_tensor_tensor(
                out=a_t[:], in0=sums[:], scalar=0.0, in1=a_t[:],
                op0=op.max, op1=op.mult,
            )
            # relu(-diff)*b  ==  min(diff,0) * (-b)
            nc.vector.scalar_tensor_tensor(
                out=b_t[:], in0=sums[:], scalar=0.0, in1=b_t[:],
                op0=op.min, op1=op.mult,
            )
            nc.vector.scalar_tensor_tensor(
                out=rhs_t[:], in0=a_t[:], scalar=0.0, in1=b_t[:],
                op0=op.add, op1=op.add, accum_out=acc[:],
            )

            ones = pp.tile([P, 1], f32)
            nc.vector.memset(ones[:], 1.0)
            ps = psp.tile([1, 1], f32)
            nc.tensor.matmul(out=ps[:], lhsT=ones[:], rhs=acc[:], start=True, stop=True)
            res = pp.tile([1, 1], f32)
            nc.vector.tensor_copy(out=res[:], in_=ps[:])
            nc.sync.dma_start(out=out_d[:], in_=res[:])

    nc.compile()
    return nc


# --------------------------------------------------------------------------
# PJRT runner (keeps the jitted executable for repeated timed runs)
# --------------------------------------------------------------------------

class _Runner:
    def __init__(self, nc):
        import jax
        import jax.numpy as jnp  # noqa: F401
        from jax.sharding import Mesh, PartitionSpec
        from jax.experimental.shard_map import shard_map
        from concourse import mybir
        from concourse.bass2jax import _bass_exec_p, install_neuronx_cc_hook

        install_neuronx_cc_hook()
        self.jax = jax

        partition_name = (
            nc.partition_id_tensor.name if nc.partition_id_tensor else None
        )
        in_names, out_names, out_avals, zero_outs = [], [], [], []
        for alloc in nc.m.functions[0].allocations:
            if not isinstance(alloc, mybir.MemoryLocationSet):
                continue
            name = alloc.memorylocations[0].name
            if alloc.kind == "ExternalInput":
                if name != partition_name:
                    in_names.append(name)
            elif alloc.kind == "ExternalOutput":
                out_names.append(name)
                shape = tuple(alloc.tensor_shape)
                dtype = mybir.dt.np(alloc.dtype)
                out_avals.append(jax.core.ShapedArray(shape, dtype))
                zero_outs.append(np.zeros(shape, dtype))
        n_params = len(in_names)
        n_outs = len(out_avals)
        all_in_names = list(in_names) + list(out_names)
        if partition_name is not None:
            all_in_names.append(partition_name)

        def _body(*args):
            operands = list(args)
            if partition_name is not None:
                from concourse.bass2jax import partition_id_tensor

                operands.append(partition_id_tensor())
            outs = _bass_exec_p.bind(
                *operands,
                out_avals=tuple(out_avals),
                in_names=tuple(all_in_names),
                out_names=tuple(out_names),
                lowering_input_output_aliases=(),
                sim_require_finite=True,
                sim_require_nnan=True,
                nc=nc,
            )
            return tuple(outs)

        devices = jax.devices()[:N_CORES]
        mesh = Mesh(np.asarray(devices), ("core",))
        self.mesh = mesh
        in_specs = (PartitionSpec("core"),) * (n_params + n_outs)
        out_specs = (PartitionSpec("core"),) * n_outs
        donate = tuple(range(n_params, n_params + n_outs))
        self.sharded = jax.jit(
            shard_map(
                _body, mesh=mesh, in_specs=in_specs, out_specs=out_specs,
                check_rep=False,
            ),
            donate_argnums=donate,
            keep_unused=True,
        )
        self.in_names = in_names
        self.out_names = out_names
        self.zero_outs = zero_outs
        self.n_params = n_params

    def stage(self, in_maps):
        """Concatenate per-core inputs and push to device, sharded by core."""
        from jax.sharding import NamedSharding, PartitionSpec

        sh = NamedSharding(self.mesh, PartitionSpec("core"))
        concat = [
            np.ascontiguousarray(
                np.concatenate([m[name] for m in in_maps], axis=0)
            )
            for name in self.in_names
        ]
        self.staged = [self.jax.device_put(a, sh) for a in concat]
        for a in self.staged:
            a.block_until_ready()

    def run(self):
        zeros = [
            np.zeros((N_CORES * z.shape[0], *z.shape[1:]), z.dtype)
            for z in self.zero_outs
        ]
        outs = self.sharded(*self.staged, *zeros)
        outs = [np.asarray(o) for o in outs]
        per_core = []
        for k in range(N_CORES):
            d = {}
            for i, name in enumerate(self.out_names):
                d[name] = outs[i].reshape(
                    N_CORES, *self.zero_outs[i].shape
                )[k]
            per_core.append(d)
        return per_core


_CACHE = {}


def _get_runner(meta_key, meta, CW, TOTCOL, dt_name, repeat=1):
    key = (meta_key, repeat)
    if key in _CACHE:
        return _CACHE[key]
    from concourse import mybir

    dt = getattr(mybir.dt, dt_name)
    nc = _build_nc(meta, CW, TOTCOL, dt, repeat=repeat)
    runner = _Runner(nc)
    _CACHE[key] = runner
    return runner


# --------------------------------------------------------------------------
# Entry point
# --------------------------------------------------------------------------

def kernel(pred, coeff, constr_rhs, var_lb, var_ub, constr_idx, var_idx,
           constr_sense, n_vars, n_constrs, **_unused):
    n_constrs = int(n_constrs)
    n_vars = int(n_vars)
    pred = np.asarray(pred)
    coeff = np.asarray(coeff)
    constr_rhs = np.asarray(constr_rhs)
    var_lb = np.asarray(var_lb)
    var_ub = np.asarray(var_ub)
    constr_idx = np.asarray(constr_idx)
    var_idx = np.asarray(var_idx)
    constr_sense = np.asarray(constr_sense)

    if DTYPE == "float32":
        dt_np = np.float32
    else:
        import ml_dtypes

        dt_np = ml_dtypes.bfloat16

    plan = _plan(constr_idx, n_constrs)
    in_maps = _build_in_maps(
        plan, pred, coeff, constr_rhs, var_lb, var_ub, constr_idx, var_idx,
        constr_sense, n_constrs, dt_np,
    )
    meta_key = (tuple(plan["meta"]), plan["CW"], plan["TOTCOL"], DTYPE)
    runner = _get_runner(meta_key, plan["meta"], plan["CW"], plan["TOTCOL"],
                         DTYPE)
    runner.stage(in_maps)
    results = runner.run()
    total = sum(float(r["out"][0, 0]) for r in results)
    return np.float32(total / n_constrs)
